# revision 19
# baseline (speedup 1.0000x reference)
"""Trainium2 Bass kernel for nn_BinaryTokenClassificationModel (segment_reduce).

Math: logits[b,i,j] = dot(segmean(1+i), w_src) + dot(segmean(513+j), w_tgt) + b,
where segmean(s) is the mean of outputs[b] over the s-th consecutive run of
equal word_ids.  dot commutes with the segment mean, so per-token projections
proj[t] = x[t]·w_c suffice.  v2 design, derived from the baseline trace:

- The kernel is HBM-DMA-bound: only tokens with segment id <= 1024 matter
  (~10.5MB/core), and the 16 per-core DMA engines cap at ~360 GB/s => ~29us
  floor.  Everything else is arranged to hide behind a saturated x stream.
- consts + w ride at the HEAD of the same sync-queue DMA stream as x, so the
  first tile's compute is never starved (baseline lost ~11us to Q10 weight
  DMAs starved behind the x backlog).
- Per-token dots use ONE fused multiply+reduce instruction per 128-token tile,
  alternating between the Vector engine (tensor_tensor_reduce) and the Pool
  engine (scalar_tensor_tensor with accum_out), each ~1us — both far under the
  2.9us/pair DMA cadence.  The scalar engine only broadcasts w and helps with
  the tail; nothing runs on the slow path anymore.
- Ragged segment-sums accumulate on the PE: per tile one tiny matmul per
  (seg-chunk u, src/tgt) PSUM column region [128,1], lhsT = on-chip-generated
  s_lo one-hot, rhs = the tile's dot column.  Host plans region start/stop.
  1/count is a host-side constant multiplied in at the tail (counts are pure
  word_ids metadata), so no count accumulation on-device.
- Tail: segment means -> s1/s2 selector matmuls (src cols) + stationary
  broadcast staircase (tgt row) -> 4 broadcast-adds -> 4 stores, as baseline.

Sharding: pure data parallel, one example (B=8) per NeuronCore (8 cores).
"""
import sys

for _p in ("/opt/trn_rl_repo", "/root/.axon_site/_ro/trn_rl_repo"):
    if _p not in sys.path:
        sys.path.append(_p)

from contextlib import ExitStack

import numpy as np

import concourse.bacc as bacc
import concourse.bass as bass
import concourse.tile as tile
from concourse import mybir
from concourse.bass_utils import run_bass_kernel_spmd

F32 = mybir.dt.float32
P = 128
H = 1024
AL = mybir.AluOpType

# pool column regions: (seg_chunk u, c) with c: 0=src (segs 1..512), 1=tgt
# (segs 513..1024).  seg s -> chunk u = s//128, slo = s%128.
REGIONS = [(0, 0), (1, 0), (2, 0), (3, 0), (4, 0),
           (4, 1), (5, 1), (6, 1), (7, 1), (8, 1)]
NREG = len(REGIONS)


def _build_nc(NT: int, plan: dict) -> bass.Bass:
    NCOL = plan["ncol"]
    KSPLIT = plan["ksplit"]
    passes = plan["passes"]        # per half-tile: list of (c, eng)
    region_mms = plan["region_mms"]  # per region q: list of (i, col, c)
    emit_after = plan["emit_after"]  # per half-tile: regions whose last tile is i
    untouched = plan["untouched"]

    nc = bacc.Bacc("TRN2", target_bir_lowering=False, debug=False, num_devices=8)
    NCC = 4 * P + NCOL + NREG + 1
    x_d = nc.declare_dram_parameter("x", [NT * P, H], F32, isOutput=False)
    cc_d = nc.declare_dram_parameter("consts", [P, NCC], F32, isOutput=False)
    w_d = nc.declare_dram_parameter("wrow", [1, 2 * H], F32, isOutput=False)
    y_d = nc.declare_dram_parameter("y", [512, 512], F32, isOutput=True)

    with tile.TileContext(nc) as tc, ExitStack() as ctx:
        consts = ctx.enter_context(tc.tile_pool(name="consts", bufs=1))
        clp = ctx.enter_context(tc.tile_pool(name="clp", bufs=1))
        xpool = ctx.enter_context(tc.tile_pool(name="xp", bufs=7))
        scrg = ctx.enter_context(tc.tile_pool(name="scrg", bufs=2))
        vpool = ctx.enter_context(tc.tile_pool(name="vp", bufs=16))
        segp = ctx.enter_context(tc.tile_pool(name="segp", bufs=1))
        opool = ctx.enter_context(tc.tile_pool(name="op", bufs=4))
        # DVE-stream scratch lives in PSUM (SBUF bandwidth is the second
        # roofline; Pool cannot touch PSUM so its scratch stays in SBUF).
        # The w-broadcast PSUM tiles share this ring via tag.
        psum_scr = ctx.enter_context(tc.tile_pool(name="pscr", bufs=2, space="PSUM"))
        ppool_acc = ctx.enter_context(tc.tile_pool(name="pacc", bufs=1, space="PSUM"))
        ppool_sm = ctx.enter_context(tc.tile_pool(name="psm", bufs=2, space="PSUM"))

        # ---- head of the sync DMA stream: consts, then w row, then x pairs.
        # Same queue => FIFO on the DMA engines => weights always beat tile 0.
        cc = consts.tile([P, NCC], F32)
        nc.sync.dma_start(out=cc, in_=cc_d[:])
        wrow = consts.tile([1, 2 * H], F32)
        nc.sync.dma_start(out=wrow, in_=w_d[:])

        ident = cc[:, 0:P]
        s1 = cc[:, P:2 * P]
        s2 = cc[:, 2 * P:3 * P]
        iota = cc[:, 3 * P:4 * P]
        slo_mat = cc[:, 4 * P:4 * P + NCOL]
        rec = cc[:, 4 * P + NCOL:4 * P + NCOL + NREG]
        biascol = cc[:, NCC - 1:NCC]

        # ---- broadcast w across partitions: ones-row stationary matmuls into
        # PSUM, then scalar-engine copies into SBUF (both engines idle here).
        ones_row = consts.tile([1, P], F32)
        nc.gpsimd.memset(ones_row, 1.0)
        wrep = consts.tile([P, 2 * H], F32)
        for q in range(2):
            pw = psum_scr.tile([P, H], F32, tag="ps", name=f"pw{q}")
            for hh in range(2):
                lo = 1024 * q + 512 * hh
                nc.tensor.matmul(pw[:, 512 * hh:512 * (hh + 1)], lhsT=ones_row,
                                 rhs=wrow[:, lo:lo + 512], start=True, stop=True)
                nc.scalar.copy(out=wrep[:, lo:lo + 512],
                               in_=pw[:, 512 * hh:512 * (hh + 1)])

        # ---- s_lo one-hots for every (half-tile, region) column, generated
        # on-chip from host slo metadata; split across Vector and Pool.
        # is_equal is a DVE-only ALU op (Pool rejects it at ISA check)
        cl_all = clp.tile([P, NCOL, P], F32)
        nc.vector.tensor_tensor(
            out=cl_all,
            in0=iota.unsqueeze(1).to_broadcast((P, NCOL, P)),
            in1=slo_mat.unsqueeze(2).to_broadcast((P, NCOL, P)),
            op=AL.is_equal)

        pool_ps = ppool_acc.tile([P, NREG], F32)
        for q in untouched:
            nc.vector.memset(pool_ps[:, q:q + 1], 0.0)

        # ---- main loop: fused dot per half-tile; each region's segment-sum
        # matmuls are emitted as ONE contiguous PE accumulation group as soon
        # as the region's last contributing tile is in (interleaved start
        # flags across open groups corrupt PSUM accumulation state).
        vmap = {}
        for g in range(NT // 2):
            x_pair = xpool.tile([P, 2, H], F32)
            src = x_d[256 * g:256 * (g + 1), :].rearrange("(two p) h -> p two h", p=P)
            nc.sync.dma_start(out=x_pair, in_=src)
            for half in range(2):
                i = 2 * g + half
                x_sub = x_pair[:, half, :]
                for (c, eng) in passes[i]:
                    v = vpool.tile([P, 1], F32, name=f"v{i}_{c}")
                    vmap[(i, c)] = v
                    # multiplies alternate DVE / Pool; the scalar engine's
                    # fused activation-accumulate does every reduction
                    # (DVE's tensor_tensor_reduce dies on real hardware)
                    if eng == "v":
                        scr = psum_scr.tile([P, H], F32, tag="ps", name="scrv")
                        nc.vector.tensor_tensor(
                            out=scr, in0=x_sub, in1=wrep[:, c * H:(c + 1) * H],
                            op=AL.mult)
                    else:
                        scr = scrg.tile([P, H], F32, name="scrg")
                        nc.gpsimd.tensor_tensor(
                            out=scr, in0=x_sub, in1=wrep[:, c * H:(c + 1) * H],
                            op=AL.mult)
                    nc.scalar.activation(
                        out=scr, in_=scr,
                        func=mybir.ActivationFunctionType.Copy,
                        accum_out=v)
                for q in emit_after[i]:
                    group = region_mms[q]
                    for n, (ti, k, c) in enumerate(group):
                        nc.tensor.matmul(pool_ps[:, q:q + 1], lhsT=cl_all[:, k, :],
                                         rhs=vmap[(ti, c)], start=(n == 0),
                                         stop=(n == len(group) - 1),
                                         skip_group_check=True)

        # ---- tail: means, src/tgt extraction, broadcast-add, stores ----
        pool_sb = segp.tile([P, NREG], F32)
        nc.vector.tensor_copy(out=pool_sb, in_=pool_ps)
        mean = segp.tile([P, NREG], F32)
        nc.vector.tensor_tensor(out=mean, in0=pool_sb, in1=rec, op=AL.mult)

        msrc_ps = ppool_sm.tile([P, 4], F32, tag="sm")
        nc.tensor.matmul(msrc_ps, lhsT=s1, rhs=mean[:, 0:4], start=True, stop=False)
        nc.tensor.matmul(msrc_ps, lhsT=s2, rhs=mean[:, 1:5], start=False, stop=True)
        msrc = segp.tile([P, 4], F32)
        nc.vector.tensor_scalar(out=msrc, in0=msrc_ps, scalar1=biascol, scalar2=None,
                                op0=AL.add)

        # rowb[p, j] = tgt mean of segment 513+j, broadcast across partitions
        rowb_ps = ppool_sm.tile([P, 512], F32, tag="sm")
        nc.tensor.matmul(rowb_ps[:, 0:127], lhsT=mean[:, 5:6].to_broadcast((P, P)),
                         rhs=ident[:, 1:128], start=True, stop=True)
        nc.tensor.matmul(rowb_ps[:, 127:255], lhsT=mean[:, 6:7].to_broadcast((P, P)),
                         rhs=ident, start=True, stop=True)
        nc.tensor.matmul(rowb_ps[:, 255:383], lhsT=mean[:, 7:8].to_broadcast((P, P)),
                         rhs=ident, start=True, stop=True)
        nc.tensor.matmul(rowb_ps[:, 383:511], lhsT=mean[:, 8:9].to_broadcast((P, P)),
                         rhs=ident, start=True, stop=True)
        nc.tensor.matmul(rowb_ps[:, 511:512], lhsT=mean[:, 9:10].to_broadcast((P, P)),
                         rhs=ident[:, 0:1], start=True, stop=True)

        for k in range(4):
            lg = opool.tile([P, 512], F32)
            if k % 2 == 0:
                nc.scalar.activation(out=lg, in_=rowb_ps,
                                     func=mybir.ActivationFunctionType.Identity,
                                     bias=msrc[:, k:k + 1], scale=1.0)
            else:
                nc.vector.tensor_scalar(out=lg, in0=rowb_ps, scalar1=msrc[:, k:k + 1],
                                        scalar2=None, op0=AL.add)
            nc.sync.dma_start(out=y_d[P * k:P * (k + 1), :], in_=lg)

    nc.compile()
    return nc


def _host_prep(inputs):
    x = np.ascontiguousarray(np.asarray(inputs["outputs"], dtype=np.float32))
    wid = np.asarray(inputs["word_ids"]).astype(np.int64)
    cw = np.asarray(inputs["classifier_w"], dtype=np.float32)
    bias = np.float32(np.asarray(inputs["classifier_b"]))
    B, L, Hd = x.shape
    assert (Hd, L) == (H, 4096) and B == 8
    assert int(inputs["num_src"]) == 512 and int(inputs["num_tgt"]) == 512

    # consecutive-run segment ids (attention_mask is all ones for this problem)
    new_seg = np.ones((B, L), np.int64)
    new_seg[:, 1:] = wid[:, 1:] != wid[:, :-1]
    seg = np.cumsum(new_seg, axis=1) - 1

    # token cutoff: segments beyond 1024 never reach the output
    cutoff = max(int(np.nonzero(seg[b] <= 1024)[0][-1]) for b in range(B))
    NPAIR = min((cutoff + 1 + 255) // 256, L // 256)
    NT = 2 * NPAIR
    Ltok = NT * P

    # per half-tile (128 contiguous tokens) region/column plan, union over cores
    segt = seg[:, :Ltok].reshape(B, NT, P)        # [B, NT, 128]
    valid = segt <= 1024
    u_of = segt // P
    c_of = (segt > 512).astype(np.int64)          # 0=src, 1=tgt
    qidx = {r: q for q, r in enumerate(REGIONS)}

    cols = []                        # (i, q) -> column index k
    region_mms = [[] for _ in REGIONS]  # per q: [(i, k, c)] ascending i
    passes = []                      # per i: list of (c, eng)
    for i in range(NT):
        regs = set()
        for b in range(B):
            vb = valid[b, i]
            if not vb.any():
                continue
            for u, c in zip(u_of[b, i][vb], c_of[b, i][vb]):
                regs.add((int(u), int(c)))
        regs = sorted(regs, key=lambda r: qidx[r])
        for r in regs:
            q = qidx[r]
            k = len(cols)
            cols.append((i, q))
            region_mms[q].append((i, k, r[1]))
        need_c = sorted({r[1] for r in regs})
        default_eng = "v" if i % 2 == 0 else "g"
        other_eng = "g" if default_eng == "v" else "v"
        pl = []
        for n, c in enumerate(need_c):
            pl.append((c, default_eng if n == 0 else other_eng))
        passes.append(pl)

    # emit each region's accumulation group right after its last tile's pass
    emit_after = [[] for _ in range(NT)]
    for q, group in enumerate(region_mms):
        if group:
            emit_after[group[-1][0]].append(q)
    NCOL = len(cols)
    untouched = [q for q in range(NREG) if not region_mms[q]]

    # per-core slo columns (-1 masks a token out of that column's region)
    # and per-core 1/count tail constants
    ident = np.eye(P, dtype=np.float32)
    s1 = np.eye(P, k=-1, dtype=np.float32)
    s2 = np.zeros((P, P), np.float32)
    s2[0, P - 1] = 1.0
    iota = np.broadcast_to(np.arange(P, dtype=np.float32), (P, P)).copy()

    in_maps = []
    for b in range(B):
        slo_mat = np.full((P, NCOL), -1.0, np.float32)
        for k, (i, q) in enumerate(cols):
            u, c = REGIONS[q]
            sel = valid[b, i] & (u_of[b, i] == u) & (c_of[b, i] == c)
            slo_mat[sel, k] = (segt[b, i][sel] % P).astype(np.float32)
        cnt = np.bincount(seg[b][seg[b] <= 1024], minlength=1152).astype(np.float32)
        recm = np.zeros((P, NREG), np.float32)
        for q, (u, c) in enumerate(REGIONS):
            s_ids = u * P + np.arange(P)
            s_c = (s_ids > 512).astype(np.int64)
            ok = (cnt[s_ids] > 0) & (s_c == c) & (s_ids <= 1024)
            recm[ok, q] = 1.0 / cnt[s_ids][ok]
        biascol = np.full((P, 1), bias, np.float32)
        cc = np.concatenate([ident, s1, s2, iota, slo_mat, recm, biascol], axis=1)
        in_maps.append({
            "x": np.ascontiguousarray(x[b, :Ltok]),
            "consts": np.ascontiguousarray(cc),
            "wrow": np.ascontiguousarray(cw.reshape(1, 2 * H)),
        })
    plan = {
        "ncol": NCOL,
        "ksplit": (NCOL + 1) // 2,
        "passes": passes,
        "region_mms": region_mms,
        "emit_after": emit_after,
        "untouched": untouched,
    }
    return NT, plan, in_maps


def _run(inputs, trace=False, tmpdir=None):
    NT, plan, in_maps = _host_prep(inputs)
    nc = _build_nc(NT, plan)
    res = run_bass_kernel_spmd(nc, in_maps, core_ids=list(range(8)), trace=trace, tmpdir=tmpdir)
    out = np.stack([np.asarray(r["y"], dtype=np.float32) for r in res.results])
    return out, res


def kernel(**inputs) -> np.ndarray:
    out, _ = _run(inputs, trace=False)
    return out


if __name__ == "__main__":
    # CoreSim smoke test on core 0's inputs
    import jax
    jax.config.update("jax_platforms", "cpu")
    sys.path.insert(0, "/root/problem")
    import reference as ref
    from concourse.bass_interp import CoreSim

    inputs = ref.setup_inputs()
    NT, plan, in_maps = _host_prep(inputs)
    print("NT =", NT, "NCOL =", plan["ncol"], "untouched:", plan["untouched"])
    npass = sum(len(p) for p in plan["passes"])
    nmm = sum(len(m) for m in plan["region_mms"])
    print("proj passes:", npass, "matmuls:", nmm)
    nc = _build_nc(NT, plan)
    sim = CoreSim(nc)
    for name, arr in in_maps[0].items():
        sim.tensor(name)[:] = arr
    sim.simulate()
    got = np.array(sim.tensor("y"))
    expected = np.asarray(ref.reference(**inputs))[0]
    err = np.abs(got - expected).max()
    scale = np.abs(expected).max()
    print("CoreSim abs err:", err, "rel:", err / scale)
    assert err / scale < 1e-2, "CoreSim mismatch"
    print("CORESIM PASSES")


# revision 24
# speedup vs baseline: 1.1513x; 1.1513x over previous
"""Trainium2 Bass kernel for nn_BinaryTokenClassificationModel (segment_reduce).

Math: logits[b,i,j] = dot(segmean(1+i), w_src) + dot(segmean(513+j), w_tgt) + b,
where segmean(s) is the mean of outputs[b] over the s-th consecutive run of
equal word_ids.  dot commutes with the segment mean, so per-token projections
proj[t] = x[t]·w_c suffice.  Design notes (from HW traces):

- HBM-DMA-bound: only tokens with segment id <= 1024 matter (~10.5MB/core);
  the 16 per-core DMA engines cap at ~360 GB/s => ~29us floor.  Everything is
  arranged to hide behind a saturated x stream on ONE queue (consts ride at
  its head so tile-0 compute is never starved).
- SBUF bandwidth is the second roofline: only the DVE multiply + scalar-engine
  fused activation-accumulate pipeline sustains full rate alongside the DMA
  stream (Pool's f32 tensor_tensor is ~4x slow and poisons DVE when co-run;
  DVE->PSUM writes are slower than SBUF; tensor_tensor_reduce and Pool's
  scalar_tensor_tensor die on real hardware).  So: DVE multiplies (1.22us),
  ACT reduces (1.07us), per 1.43us DMA tile slot.
- Ragged segment-sums accumulate on the PE: per (seg-chunk u, src/tgt) PSUM
  column region, one contiguous accumulation group of tiny [128,1]-rhs
  matmuls, emitted when the region's last contributing tile is in
  (interleaved start flags across open PSUM groups corrupt accumulation).
  lhsT = on-chip-generated s_lo one-hots; 1/count is folded in host-side
  constants at the tail (counts are pure word_ids metadata).
- Tail is split: src-side means + selector matmuls emit mid-loop; only the
  tgt broadcast staircase + 4 adds + stores trail the last tile.

Sharding: pure data parallel, one example (B=8) per NeuronCore (8 cores).
"""
import sys

for _p in ("/opt/trn_rl_repo", "/root/.axon_site/_ro/trn_rl_repo"):
    if _p not in sys.path:
        sys.path.append(_p)

from contextlib import ExitStack

import numpy as np

import concourse.bacc as bacc
import concourse.bass as bass
import concourse.tile as tile
from concourse import mybir
from concourse.bass_utils import run_bass_kernel_spmd

F32 = mybir.dt.float32
P = 128
H = 1024
AL = mybir.AluOpType

# pool column regions: (seg_chunk u, c) with c: 0=src (segs 1..512), 1=tgt
# (segs 513..1024).  seg s -> chunk u = s//128, slo = s%128.
REGIONS = [(0, 0), (1, 0), (2, 0), (3, 0), (4, 0),
           (4, 1), (5, 1), (6, 1), (7, 1), (8, 1)]
NREG = len(REGIONS)
Q_SRC_LAST = 4   # regions 0..4 feed the msrc tail piece


def _build_nc(NT: int, plan: dict) -> bass.Bass:
    NCOL = plan["ncol"]
    KSPLIT = plan["ksplit"]          # cl columns needed by the first pair
    passes = plan["passes"]          # per half-tile: list of c values
    region_mms = plan["region_mms"]  # per region q: list of (i, col, c)
    emit_after = plan["emit_after"]  # per half-tile: regions whose last tile is i
    untouched = plan["untouched"]

    nc = bacc.Bacc("TRN2", target_bir_lowering=False, debug=False, num_devices=8)
    NCE = P + NCOL + NREG + 1        # early consts: iota | slo | rec | bias
    x_d = nc.declare_dram_parameter("x", [NT * P, H], F32, isOutput=False)
    ce_d = nc.declare_dram_parameter("consts", [P, NCE], F32, isOutput=False)
    ct_d = nc.declare_dram_parameter("consts_tail", [P, 3 * P], F32, isOutput=False)
    w_d = nc.declare_dram_parameter("wrow", [1, 2 * H], F32, isOutput=False)
    y_d = nc.declare_dram_parameter("y", [512, 512], F32, isOutput=True)

    with tile.TileContext(nc) as tc, ExitStack() as ctx:
        consts = ctx.enter_context(tc.tile_pool(name="consts", bufs=1))
        clp = ctx.enter_context(tc.tile_pool(name="clp", bufs=1))
        xpool = ctx.enter_context(tc.tile_pool(name="xp", bufs=7))
        scrv = ctx.enter_context(tc.tile_pool(name="scrv", bufs=2))
        vpool = ctx.enter_context(tc.tile_pool(name="vp", bufs=16))
        segp = ctx.enter_context(tc.tile_pool(name="segp", bufs=1))
        opool = ctx.enter_context(tc.tile_pool(name="op", bufs=4))
        pw_pool = ctx.enter_context(tc.tile_pool(name="pw", bufs=4, space="PSUM"))
        ppool_acc = ctx.enter_context(tc.tile_pool(name="pacc", bufs=1, space="PSUM"))
        ppool_sm = ctx.enter_context(tc.tile_pool(name="psm", bufs=2, space="PSUM"))

        # ---- head of the sync DMA stream: early consts, w row, then x pairs
        # (same queue => FIFO on the DMA engines => nothing starves tile 0);
        # tail-only consts (ident/s1/s2 selectors) are queued BEHIND all x.
        cc = consts.tile([P, NCE], F32)
        nc.sync.dma_start(out=cc, in_=ce_d[:])
        wrow = consts.tile([1, 2 * H], F32)
        nc.sync.dma_start(out=wrow, in_=w_d[:])

        iota = cc[:, 0:P]
        slo_mat = cc[:, P:P + NCOL]
        rec = cc[:, P + NCOL:P + NCOL + NREG]
        biascol = cc[:, NCE - 1:NCE]
        ct = consts.tile([P, 3 * P], F32)
        ident = ct[:, 0:P]
        s1 = ct[:, P:2 * P]
        s2 = ct[:, 2 * P:3 * P]

        # ---- broadcast w across partitions: ones-row stationary matmuls into
        # PSUM, then scalar-engine copies into SBUF (idle engines at start).
        ones_row = consts.tile([1, P], F32)
        nc.gpsimd.memset(ones_row, 1.0)
        wrep = consts.tile([P, 2 * H], F32)
        for q in range(4):
            pw = pw_pool.tile([P, 512], F32, tag="pw", name=f"pw{q}")
            nc.tensor.matmul(pw, lhsT=ones_row, rhs=wrow[:, 512 * q:512 * (q + 1)],
                             start=True, stop=True)
            nc.scalar.copy(out=wrep[:, 512 * q:512 * (q + 1)], in_=pw)

        # ---- s_lo one-hots, generated on DVE (is_equal is DVE-only); split
        # so the first pair's columns are ready before pair 0 lands.
        cl_all = clp.tile([P, NCOL, P], F32)
        nc.vector.tensor_tensor(
            out=cl_all[:, 0:KSPLIT],
            in0=iota.unsqueeze(1).to_broadcast((P, KSPLIT, P)),
            in1=slo_mat[:, 0:KSPLIT].unsqueeze(2).to_broadcast((P, KSPLIT, P)),
            op=AL.is_equal)
        if KSPLIT < NCOL:
            nc.vector.tensor_tensor(
                out=cl_all[:, KSPLIT:NCOL],
                in0=iota.unsqueeze(1).to_broadcast((P, NCOL - KSPLIT, P)),
                in1=slo_mat[:, KSPLIT:NCOL].unsqueeze(2).to_broadcast((P, NCOL - KSPLIT, P)),
                op=AL.is_equal)

        pool_ps = ppool_acc.tile([P, NREG], F32)
        for q in untouched:
            nc.vector.memset(pool_ps[:, q:q + 1], 0.0)

        mean = segp.tile([P, NREG], F32)
        msrc_ps = ppool_sm.tile([P, 4], F32, tag="sm")
        msrc = segp.tile([P, 4], F32)

        def emit_src_tail():
            # segs 1..512: means -> s1/s2 selector matmuls -> +bias, all
            # mid-loop (regions 0..4 close around tile NT/2)
            nc.vector.tensor_copy(out=mean[:, 0:5], in_=pool_ps[:, 0:5])
            nc.vector.tensor_tensor(out=mean[:, 0:5], in0=mean[:, 0:5],
                                    in1=rec[:, 0:5], op=AL.mult)
            nc.tensor.matmul(msrc_ps, lhsT=s1, rhs=mean[:, 0:4], start=True, stop=False)
            nc.tensor.matmul(msrc_ps, lhsT=s2, rhs=mean[:, 1:5], start=False, stop=True)
            nc.vector.tensor_scalar(out=msrc, in0=msrc_ps, scalar1=biascol,
                                    scalar2=None, op0=AL.add)

        # ---- main loop: DVE multiply + ACT fused reduce per half-tile; each
        # region's segment-sum matmuls form ONE contiguous PE accumulation
        # group, emitted when its last contributing tile is in.
        vmap = {}
        src_tail_done = False
        for g in range(NT // 2):
            x_pair = xpool.tile([P, 2, H], F32, name="x_pair", tag="x_pair")
            src = x_d[256 * g:256 * (g + 1), :].rearrange("(two p) h -> p two h", p=P)
            nc.sync.dma_start(out=x_pair, in_=src)
            if g == min(4, NT // 2 - 1):
                # tail-only selector consts: behind 5 pairs of x (keeps the
                # head tight), well before the mid-loop src tail needs them
                nc.sync.dma_start(out=ct, in_=ct_d[:])
            for half in range(2):
                i = 2 * g + half
                x_sub = x_pair[:, half, :]
                for c in passes[i]:
                    v = vpool.tile([P, 1], F32, name=f"v{i}_{c}")
                    vmap[(i, c)] = v
                    scr = scrv.tile([P, H], F32, name="scrv")
                    nc.vector.tensor_tensor(
                        out=scr, in0=x_sub, in1=wrep[:, c * H:(c + 1) * H],
                        op=AL.mult)
                    nc.scalar.activation(
                        out=scr, in_=scr,
                        func=mybir.ActivationFunctionType.Copy,
                        accum_out=v)
                for q in emit_after[i]:
                    group = region_mms[q]
                    for n, (ti, k, c) in enumerate(group):
                        nc.tensor.matmul(pool_ps[:, q:q + 1], lhsT=cl_all[:, k, :],
                                         rhs=vmap[(ti, c)], start=(n == 0),
                                         stop=(n == len(group) - 1),
                                         skip_group_check=True)
                if not src_tail_done and emit_after[i] and max(emit_after[i]) >= Q_SRC_LAST:
                    emit_src_tail()
                    src_tail_done = True
        if not src_tail_done:
            emit_src_tail()

        # ---- tail: tgt means, broadcast staircase, 4 adds, stores ----
        nc.vector.tensor_copy(out=mean[:, 5:10], in_=pool_ps[:, 5:10])
        nc.vector.tensor_tensor(out=mean[:, 5:10], in0=mean[:, 5:10],
                                in1=rec[:, 5:10], op=AL.mult)

        # rowb[p, j] = tgt mean of segment 513+j, broadcast across partitions
        rowb_ps = ppool_sm.tile([P, 512], F32, tag="sm")
        nc.tensor.matmul(rowb_ps[:, 0:127], lhsT=mean[:, 5:6].to_broadcast((P, P)),
                         rhs=ident[:, 1:128], start=True, stop=True)
        nc.tensor.matmul(rowb_ps[:, 127:255], lhsT=mean[:, 6:7].to_broadcast((P, P)),
                         rhs=ident, start=True, stop=True)
        nc.tensor.matmul(rowb_ps[:, 255:383], lhsT=mean[:, 7:8].to_broadcast((P, P)),
                         rhs=ident, start=True, stop=True)
        nc.tensor.matmul(rowb_ps[:, 383:511], lhsT=mean[:, 8:9].to_broadcast((P, P)),
                         rhs=ident, start=True, stop=True)
        nc.tensor.matmul(rowb_ps[:, 511:512], lhsT=mean[:, 9:10].to_broadcast((P, P)),
                         rhs=ident[:, 0:1], start=True, stop=True)

        for k in range(4):
            lg = opool.tile([P, 512], F32, name=f"lg{k}")
            if k % 2 == 0:
                nc.scalar.activation(out=lg, in_=rowb_ps,
                                     func=mybir.ActivationFunctionType.Identity,
                                     bias=msrc[:, k:k + 1], scale=1.0)
            else:
                nc.vector.tensor_scalar(out=lg, in0=rowb_ps, scalar1=msrc[:, k:k + 1],
                                        scalar2=None, op0=AL.add)
            nc.sync.dma_start(out=y_d[P * k:P * (k + 1), :], in_=lg)

    nc.compile()
    return nc


def _host_prep(inputs):
    x = np.ascontiguousarray(np.asarray(inputs["outputs"], dtype=np.float32))
    wid = np.asarray(inputs["word_ids"]).astype(np.int64)
    cw = np.asarray(inputs["classifier_w"], dtype=np.float32)
    bias = np.float32(np.asarray(inputs["classifier_b"]))
    B, L, Hd = x.shape
    assert (Hd, L) == (H, 4096) and B == 8
    assert int(inputs["num_src"]) == 512 and int(inputs["num_tgt"]) == 512

    # consecutive-run segment ids (attention_mask is all ones for this problem)
    new_seg = np.ones((B, L), np.int64)
    new_seg[:, 1:] = wid[:, 1:] != wid[:, :-1]
    seg = np.cumsum(new_seg, axis=1) - 1

    # token cutoff: segments beyond 1024 never reach the output
    cutoff = max(int(np.nonzero(seg[b] <= 1024)[0][-1]) for b in range(B))
    NPAIR = min((cutoff + 1 + 255) // 256, L // 256)
    NT = 2 * NPAIR
    Ltok = NT * P

    # per half-tile (128 contiguous tokens) region/column plan, union over cores
    segt = seg[:, :Ltok].reshape(B, NT, P)        # [B, NT, 128]
    valid = segt <= 1024
    u_of = segt // P
    c_of = (segt > 512).astype(np.int64)          # 0=src, 1=tgt
    qidx = {r: q for q, r in enumerate(REGIONS)}

    cols = []                           # (i, q) -> column index k
    region_mms = [[] for _ in REGIONS]  # per q: [(i, k, c)] ascending i
    passes = []                         # per i: list of c values
    for i in range(NT):
        regs = set()
        for b in range(B):
            vb = valid[b, i]
            if not vb.any():
                continue
            for u, c in zip(u_of[b, i][vb], c_of[b, i][vb]):
                regs.add((int(u), int(c)))
        regs = sorted(regs, key=lambda r: qidx[r])
        for r in regs:
            q = qidx[r]
            k = len(cols)
            cols.append((i, q))
            region_mms[q].append((i, k, r[1]))
        passes.append(sorted({r[1] for r in regs}))

    # emit each region's accumulation group right after its last tile's pass
    emit_after = [[] for _ in range(NT)]
    for q, group in enumerate(region_mms):
        if group:
            emit_after[group[-1][0]].append(q)
    NCOL = len(cols)
    untouched = [q for q in range(NREG) if not region_mms[q]]
    # cl columns the first pair needs (generated in the first is_equal op)
    ksplit = max([k + 1 for k, (i, q) in enumerate(cols) if i < 4] or [1])

    # per-core slo columns (-1 masks a token out of that column's region)
    # and per-core 1/count tail constants
    ident = np.eye(P, dtype=np.float32)
    s1 = np.eye(P, k=-1, dtype=np.float32)
    s2 = np.zeros((P, P), np.float32)
    s2[0, P - 1] = 1.0
    iota = np.broadcast_to(np.arange(P, dtype=np.float32), (P, P)).copy()
    ct = np.ascontiguousarray(np.concatenate([ident, s1, s2], axis=1))

    in_maps = []
    for b in range(B):
        slo_mat = np.full((P, NCOL), -1.0, np.float32)
        for k, (i, q) in enumerate(cols):
            u, c = REGIONS[q]
            sel = valid[b, i] & (u_of[b, i] == u) & (c_of[b, i] == c)
            slo_mat[sel, k] = (segt[b, i][sel] % P).astype(np.float32)
        cnt = np.bincount(seg[b][seg[b] <= 1024], minlength=1152).astype(np.float32)
        recm = np.zeros((P, NREG), np.float32)
        for q, (u, c) in enumerate(REGIONS):
            s_ids = u * P + np.arange(P)
            s_c = (s_ids > 512).astype(np.int64)
            ok = (cnt[s_ids] > 0) & (s_c == c) & (s_ids <= 1024)
            recm[ok, q] = 1.0 / cnt[s_ids][ok]
        biascol = np.full((P, 1), bias, np.float32)
        cc = np.concatenate([iota, slo_mat, recm, biascol], axis=1)
        in_maps.append({
            "x": np.ascontiguousarray(x[b, :Ltok]),
            "consts": np.ascontiguousarray(cc),
            "consts_tail": ct,
            "wrow": np.ascontiguousarray(cw.reshape(1, 2 * H)),
        })
    plan = {
        "ncol": NCOL,
        "ksplit": ksplit,
        "passes": passes,
        "region_mms": region_mms,
        "emit_after": emit_after,
        "untouched": untouched,
    }
    return NT, plan, in_maps


def _run(inputs, trace=False, tmpdir=None):
    NT, plan, in_maps = _host_prep(inputs)
    nc = _build_nc(NT, plan)
    res = run_bass_kernel_spmd(nc, in_maps, core_ids=list(range(8)), trace=trace, tmpdir=tmpdir)
    out = np.stack([np.asarray(r["y"], dtype=np.float32) for r in res.results])
    return out, res


def kernel(**inputs) -> np.ndarray:
    out, _ = _run(inputs, trace=False)
    return out


if __name__ == "__main__":
    # CoreSim smoke test on core 0's inputs
    import jax
    jax.config.update("jax_platforms", "cpu")
    sys.path.insert(0, "/root/problem")
    import reference as ref
    from concourse.bass_interp import CoreSim

    inputs = ref.setup_inputs()
    NT, plan, in_maps = _host_prep(inputs)
    print("NT =", NT, "NCOL =", plan["ncol"], "ksplit =", plan["ksplit"],
          "untouched:", plan["untouched"])
    npass = sum(len(p) for p in plan["passes"])
    nmm = sum(len(m) for m in plan["region_mms"])
    print("proj passes:", npass, "matmuls:", nmm)
    nc = _build_nc(NT, plan)
    sim = CoreSim(nc)
    for name, arr in in_maps[0].items():
        sim.tensor(name)[:] = arr
    sim.simulate()
    got = np.array(sim.tensor("y"))
    expected = np.asarray(ref.reference(**inputs))[0]
    err = np.abs(got - expected).max()
    scale = np.abs(expected).max()
    print("CoreSim abs err:", err, "rel:", err / scale)
    assert err / scale < 1e-2, "CoreSim mismatch"
    print("CORESIM PASSES")
